# revision 1
# baseline (speedup 1.0000x reference)
"""GIN message-passing (CentralityChannel) on 8 trn2 NeuronCores.

Strategy (graph/data parallel per sharding hint):
  - Nodes sharded: core c owns rows [c*6250, (c+1)*6250), padded to 6272 = 49
    tiles of 128. The full node-feature table h [50176, 64] is replicated in
    every core's DRAM and rebuilt each layer with an AllGather.
  - Edges sharded by TARGET core. Per target tile (128 nodes), edges are
    grouped into blocks of 128, split by source-table half (dma_gather uses
    int16 indices < 32768), zero-weight padded to static block counts
    (BLO lo-blocks + BHI hi-blocks per tile, same for every core).
  - Per block: gathered rows [128e, 64f] are weighted into bf16 copies
    (DVE), and PE accumulates aggT[64f, 128n] += msg.T @ S in PSUM ==
    segment-sum of w*h[src]. The one-hot S[e, n] = (tgtloc[e] == n) is
    static across layers, so it is precomputed on the host in bf16 and
    streamed from DRAM per 4-tile group (HWDGE), freeing DVE; bf16
    operands run the agg matmuls at 4x fp32 PE rate.
  - GIN combine is folded: edge weights are pre-divided by (1+eps_l) on the
    host and W1_l is pre-multiplied by (1+eps_l), so y0 = hT_own + aggT.
  - MLP runs feature-major: y1T = W1'.T @ y0, batched 4 tiles (512 node
    cols) per PSUM bank so ACT/DVE/PE fixed overheads amortize. BatchNorm1d
    (training mode) needs global batch stats: per-channel sum/sumsq
    accumulate via ACT accum_out, then an AllGather of the 8 partial [64,2]
    blocks + on-chip reduce (cheaper than AllReduce in the collective cost
    model), then BN+bias+ReLU is one ACT op with per-partition scale/bias.
    MLP biases cancel inside BatchNorm and are dropped.
  - New h shard is transposed back node-major via PE and AllGathered into the
    next layer's table. The final layer skips the AllGather; the host
    assembles shards and applies mask_teams.
"""

import sys

sys.path.insert(0, "/opt/trn_rl_repo")
import numpy as np

NODES, D, NL = 50000, 64, 3
NCORES = 8
NSH = NODES // NCORES            # 6250
P = 128
NT = (NSH + P - 1) // P          # 49
NLOC = NT * P                    # 6272
TB = NCORES * NLOC               # 50176
HALF = TB // 2                   # 25088
BN_EPS = 1e-5
G = 4                            # target tiles per dma_gather call
CHUNK = 512                      # node cols per MLP2 matmul

_CALLS = [list(range(s, min(s + G, NT))) for s in range(0, NT, G)]


def _plan(edge_index, edge_weight, eps):
    """Host preprocessing: shard/sort/pad edges into the static block layout."""
    src = edge_index[0].astype(np.int64)
    tgt = edge_index[1].astype(np.int64)
    w = edge_weight.astype(np.float32)
    assert np.all(np.abs(1.0 + eps) > 1e-6), "eps == -1 unsupported"

    src_row = (src // NSH) * NLOC + (src % NSH)     # row in padded table
    c_tgt = tgt // NSH
    r = tgt % NSH
    tile = r // P
    lane = (r % P).astype(np.float32)               # one-hot lane value
    half = (src_row >= HALF).astype(np.int64)

    key = (c_tgt * NT + tile) * 2 + half
    order = np.argsort(key, kind="stable")
    counts = np.bincount(key, minlength=NCORES * NT * 2)
    starts = np.zeros_like(counts)
    starts[1:] = np.cumsum(counts)[:-1]
    q = np.arange(len(src)) - starts[key[order]]    # rank within group

    BLO = int(np.ceil(counts.reshape(-1, 2)[:, 0].max() / P))
    BHI = int(np.ceil(counts.reshape(-1, 2)[:, 1].max() / P))
    BPT = BLO + BHI

    so, ho, co, to = src_row[order], half[order], c_tgt[order], tile[order]
    lo_, wo = lane[order], w[order]
    b = q // P
    p = q % P
    bcount = np.where(ho == 0, BLO, BHI)
    assert np.all(b < bcount), "block overflow; BLO/BHI too small"

    cores = []
    for c in range(NCORES):
        m = co == c
        tokv = {0: np.zeros(NT * BLO * P, np.int64),
                1: np.zeros(NT * BHI * P, np.int64)}
        wtok = {0: np.zeros(NT * BLO * P, np.float32),
                1: np.zeros(NT * BHI * P, np.float32)}
        ttok = np.zeros(NT * BPT * P, np.float32)
        for h, bp in ((0, BLO), (1, BHI)):
            mh = m & (ho == h)
            pos = (to[mh] * bp + b[mh]) * P + p[mh]
            tokv[h][pos] = so[mh] - h * HALF
            wtok[h][pos] = wo[mh]
            bb = b[mh] + (BLO if h else 0)
            ttok[(to[mh] * BPT + bb) * P + p[mh]] = lo_[mh]

        def wrap(tv, bp):
            outs = []
            for tiles in _CALLS:
                t0, t1 = tiles[0], tiles[-1] + 1
                seg = tv[t0 * bp * P:t1 * bp * P]
                outs.append(seg.reshape(-1, 16).T)
            wr = np.concatenate(outs, axis=1).astype(np.int16)
            return np.tile(wr, (8, 1))              # replicate to 128 rows

        wlo = np.stack([wtok[0] / (1.0 + eps[l]) for l in range(NL)])
        whi = np.stack([wtok[1] / (1.0 + eps[l]) for l in range(NL)])
        import ml_dtypes
        lanes = ttok.reshape(NT * BPT, P)           # [blk, p] lane values
        one_hot = (lanes[:, :, None] ==
                   np.arange(P, dtype=np.float32)[None, None, :])
        S_bf = np.ascontiguousarray(
            one_hot.transpose(1, 0, 2).reshape(P, NT * BPT * P)
        ).astype(ml_dtypes.bfloat16)                # [p, blk*n]
        cores.append(dict(
            idxlo=wrap(tokv[0], BLO), idxhi=wrap(tokv[1], BHI),
            wlo=np.concatenate([a.reshape(NT * BLO, P).T for a in wlo], 1),
            whi=np.concatenate([a.reshape(NT * BHI, P).T for a in whi], 1),
            tgtloc=ttok.reshape(NT * BPT, P).T.copy(), S=S_bf,
            tokv=tokv, wtok=wtok, ttokf=ttok,
        ))
    return dict(BLO=BLO, BHI=BHI, BPT=BPT, cores=cores)


def _tableize(x):
    rows = (np.arange(NODES) // NSH) * NLOC + np.arange(NODES) % NSH
    tb = np.zeros((TB, D), np.float32)
    tb[rows] = x
    return tb, rows


def _weights(eps, W1, W2, g1, beta1, g2, beta2):
    ws = []
    for l in range(NL):
        ws.append(((1.0 + eps[l]) * W1[l]).astype(np.float32))
        ws.append(W2[l].astype(np.float32))
    Ws = np.concatenate(ws, 0)                       # [NL*2*64, 64]
    gb = np.stack(sum([[g1[l], beta1[l], g2[l], beta2[l]] for l in range(NL)],
                      []), 1).astype(np.float32)     # [64, 12]
    return Ws, gb


def mirror(x, edge_index, edge_weight, mask_teams, eps, W1, b1, g1, beta1,
           W2, b2, g2, beta2):
    """Numpy mirror of the exact device computation (for validation)."""
    plan = _plan(np.asarray(edge_index), np.asarray(edge_weight),
                 np.asarray(eps))
    BLO, BHI, BPT = plan["BLO"], plan["BHI"], plan["BPT"]
    table, rows = _tableize(np.asarray(x))
    Ws, gb = _weights(eps, W1, W2, g1, beta1, g2, beta2)
    H = [table[c * NLOC:(c + 1) * NLOC].T.copy() for c in range(NCORES)]

    for l in range(NL):
        aggs = []
        for c in range(NCORES):
            pc = plan["cores"][c]
            msil = table[:HALF][pc["tokv"][0]] * \
                (pc["wtok"][0] / (1 + eps[l]))[:, None]
            msih = table[HALF:][pc["tokv"][1]] * \
                (pc["wtok"][1] / (1 + eps[l]))[:, None]
            mlo = msil.reshape(NT, BLO, P, D)
            mhi = msih.reshape(NT, BHI, P, D)
            msg = np.concatenate([mlo, mhi], 1)       # [NT, BPT, P, D]
            tl = pc["ttokf"].reshape(NT, BPT, P)
            S = (tl[..., None] == np.arange(P, dtype=np.float32)).astype(
                np.float32)                           # [NT, BPT, P, Pn]
            agg = np.einsum("tbpd,tbpn->dtn", msg, S).reshape(D, NLOC)
            aggs.append(agg)
        # MLP, feature-major, with cross-core BN
        y1s = []
        for c in range(NCORES):
            y0 = H[c] + aggs[c]
            y1s.append(Ws[2 * l * D:(2 * l + 1) * D].T @ y0)
        s1 = sum(y[:, :NSH].sum(1) for y in y1s)
        s1q = sum((y[:, :NSH] ** 2).sum(1) for y in y1s)
        mu, ex2 = s1 / NODES, s1q / NODES
        sc1 = gb[:, 4 * l + 0] / np.sqrt(ex2 - mu ** 2 + BN_EPS)
        bi1 = gb[:, 4 * l + 1] - mu * sc1
        y2s = []
        for c in range(NCORES):
            y1n = np.zeros_like(y1s[c])
            y1n[:, :NSH] = np.maximum(
                y1s[c][:, :NSH] * sc1[:, None] + bi1[:, None], 0)
            y2s.append(Ws[(2 * l + 1) * D:(2 * l + 2) * D].T @ y1n)
        s2 = sum(y[:, :NSH].sum(1) for y in y2s)
        s2q = sum((y[:, :NSH] ** 2).sum(1) for y in y2s)
        mu2, ex22 = s2 / NODES, s2q / NODES
        sc2 = gb[:, 4 * l + 2] / np.sqrt(ex22 - mu2 ** 2 + BN_EPS)
        bi2 = gb[:, 4 * l + 3] - mu2 * sc2
        for c in range(NCORES):
            hn = np.zeros_like(y2s[c])
            hn[:, :NSH] = np.maximum(
                y2s[c][:, :NSH] * sc2[:, None] + bi2[:, None], 0)
            H[c] = hn
            table[c * NLOC:(c + 1) * NLOC] = hn.T
    full = np.concatenate([H[c].T[:NSH] for c in range(NCORES)], 0)
    return full[np.asarray(mask_teams)]


# ---------------------------------------------------------------------------
# Device program
# ---------------------------------------------------------------------------
_cache = {}


def _build(BLO, BHI, stage=5, hi_queue=1):
    from concourse import bass, bacc, mybir, tile
    from concourse.masks import make_identity

    F = mybir.dt.float32
    BF = mybir.dt.bfloat16
    I16 = mybir.dt.int16
    BPT = BLO + BHI
    AL = mybir.AluOpType
    AF = mybir.ActivationFunctionType

    nc = bacc.Bacc(num_devices=NCORES, num_swdge_queues=2)
    x_table = nc.declare_dram_parameter("x_table", [TB, D], F, isOutput=False)
    xT_own = nc.declare_dram_parameter("xT_own", [D, NLOC], F, isOutput=False)
    idxlo = nc.declare_dram_parameter("idxlo", [P, NT * BLO * 8], I16, False)
    idxhi = nc.declare_dram_parameter("idxhi", [P, NT * BHI * 8], I16, False)
    wlo_in = nc.declare_dram_parameter("wlo", [P, NL * NT * BLO], F, False)
    whi_in = nc.declare_dram_parameter("whi", [P, NL * NT * BHI], F, False)
    S_in = nc.declare_dram_parameter("S", [P, NT * BPT * P], BF, False)
    Ws_in = nc.declare_dram_parameter("Ws", [NL * 2 * D, D], F, False)
    gb_in = nc.declare_dram_parameter("gb", [D, 4 * NL], F, False)
    h_out = nc.declare_dram_parameter("h_out", [NLOC, D], F, isOutput=True)

    cc_in = [nc.dram_tensor(f"cc_in{l}", [NLOC, D], F) for l in range(NL - 1)]
    cc_out = [nc.dram_tensor(f"cc_out{l}", [TB, D], F, addr_space="Shared")
              for l in range(NL - 1)]
    st_in = [nc.dram_tensor(f"st_in{i}", [D, 2], F) for i in range(2 * NL)]
    st_out = [nc.dram_tensor(f"st_out{i}", [NCORES * D, 2], F,
                             addr_space="Shared") for i in range(2 * NL)]
    rg = [list(range(NCORES))]

    with tile.TileContext(nc) as tc:
        with (
            tc.tile_pool(name="persist", bufs=1) as pp,
            tc.tile_pool(name="gat", bufs=2) as gp,
            tc.tile_pool(name="sg", bufs=2) as sgp,
            tc.tile_pool(name="sb", bufs=2) as sb,
            tc.tile_pool(name="small", bufs=4) as sp,
            tc.tile_pool(name="ps_agg", bufs=2, space="PSUM") as ps_agg,
            tc.tile_pool(name="ps_m", bufs=2, space="PSUM") as ps_m,
            tc.tile_pool(name="ps_m2", bufs=2, space="PSUM") as ps_m2,
            tc.tile_pool(name="ps_tr", bufs=2, space="PSUM") as ps_tr,
        ):
            # resident tiles
            H = pp.tile([D, NLOC], F)
            B1 = pp.tile([D, NLOC], F)
            B2 = pp.tile([D, NLOC], F)
            ilo = pp.tile([P, NT * BLO * 8], I16)
            ihi = pp.tile([P, NT * BHI * 8], I16)
            wlo = pp.tile([P, NL * NT * BLO], F)
            whi = pp.tile([P, NL * NT * BHI], F)
            gb = pp.tile([D, 4 * NL], F)
            ident = pp.tile([D, D], F)

            nc.sync.dma_start(out=H[:], in_=xT_own[:])
            nc.sync.dma_start(out=ilo[:], in_=idxlo[:])
            nc.sync.dma_start(out=ihi[:], in_=idxhi[:])
            nc.sync.dma_start(out=wlo[:], in_=wlo_in[:])
            nc.sync.dma_start(out=whi[:], in_=whi_in[:])
            nc.sync.dma_start(out=gb[:], in_=gb_in[:])
            make_identity(nc, ident[:])
            nc.vector.memset(B1[:], 0.0)
            nc.vector.memset(B2[:], 0.0)
            epsc = pp.tile([D, 1], F)
            nc.vector.memset(epsc[:], BN_EPS)

            NW = NSH - (NT - 1) * P                  # 106 real cols, last tile

            for l in range(NL):
                tab = x_table if l == 0 else cc_out[l - 1]
                W1t = sp.tile([D, D], F, tag="w1")
                W2t = sp.tile([D, D], F, tag="w2")
                nc.sync.dma_start(out=W1t[:], in_=Ws_in[2 * l * D:(2 * l + 1) * D, :])
                nc.sync.dma_start(out=W2t[:], in_=Ws_in[(2 * l + 1) * D:(2 * l + 2) * D, :])
                NG = len(_CALLS)
                s1 = sp.tile([D, NG], F, tag="s1")
                s1q = sp.tile([D, NG], F, tag="s1q")

                for ci, tiles in enumerate(_CALLS):
                    ntl = len(tiles)
                    t0 = tiles[0]
                    glo = gp.tile([P, G * BLO, D], F, tag="glo")
                    ghi = gp.tile([P, G * BHI, D], F, tag="ghi")
                    if stage < 1:
                        continue
                    Sg = sgp.tile([P, G * BPT, P], BF, tag="Sg")
                    nc.sync.dma_start(
                        out=Sg[:, :ntl * BPT, :],
                        in_=S_in[:, t0 * BPT * P:(t0 + ntl) * BPT * P])
                    nc.gpsimd.dma_gather(
                        out_ap=glo[:, :ntl * BLO, :], in_ap=tab[0:HALF, :],
                        idxs_ap=ilo[:, t0 * BLO * 8:(t0 + ntl) * BLO * 8],
                        num_idxs=ntl * BLO * P, num_idxs_reg=ntl * BLO * P,
                        elem_size=D, single_packet=False)
                    nc.gpsimd.dma_gather(
                        out_ap=ghi[:, :ntl * BHI, :], in_ap=tab[HALF:TB, :],
                        idxs_ap=ihi[:, t0 * BHI * 8:(t0 + ntl) * BHI * 8],
                        num_idxs=ntl * BHI * P, num_idxs_reg=ntl * BHI * P,
                        elem_size=D, single_packet=False, queue_num=hi_queue)
                    # weight the messages into bf16 copies (PE runs 4x on bf16)
                    if stage < 2:
                        continue
                    glob = gp.tile([P, G * BLO, D], BF, tag="glob")
                    ghib = gp.tile([P, G * BHI, D], BF, tag="ghib")
                    nc.vector.tensor_tensor(
                        out=glob[:, :ntl * BLO, :], in0=glo[:, :ntl * BLO, :],
                        in1=wlo[:, (l * NT + t0) * BLO:(l * NT + t0 + ntl) * BLO]
                        .to_broadcast([P, ntl * BLO, D]),
                        op=AL.mult)
                    nc.vector.tensor_tensor(
                        out=ghib[:, :ntl * BHI, :], in0=ghi[:, :ntl * BHI, :],
                        in1=whi[:, (l * NT + t0) * BHI:(l * NT + t0 + ntl) * BHI]
                        .to_broadcast([P, ntl * BHI, D]),
                        op=AL.mult)

                    if stage < 3:
                        continue
                    if stage < 4:
                        continue
                    paG = ps_agg.tile([D, G * P], F, space="PSUM", tag="pa")
                    for ti, t in enumerate(tiles):
                        for b in range(BPT):
                            if b < BLO:
                                msg = glob[:, ti * BLO + b, :]
                            else:
                                msg = ghib[:, ti * BHI + (b - BLO), :]
                            nc.tensor.matmul(
                                out=paG[:, ti * P:(ti + 1) * P], lhsT=msg,
                                rhs=Sg[:, ti * BPT + b, :],
                                start=(b == 0), stop=(b == BPT - 1))
                    # combine + MLP1, one shot per 4-tile group (pads are
                    # exactly zero in H and agg, so stats over them are safe)
                    y0G = sb.tile([D, G * P], F, tag="y0")
                    nc.vector.tensor_tensor(
                        out=y0G[:, :ntl * P], in0=paG[:, :ntl * P],
                        in1=H[:, t0 * P:(t0 + ntl) * P], op=AL.add)
                    pmG = ps_m.tile([D, G * P], F, space="PSUM", tag="pm")
                    nc.tensor.matmul(out=pmG[:, :ntl * P], lhsT=W1t[:],
                                     rhs=y0G[:, :ntl * P],
                                     start=True, stop=True)
                    sqG = sb.tile([D, G * P], F, tag="sq")
                    nc.scalar.activation(
                        out=B1[:, t0 * P:(t0 + ntl) * P],
                        in_=pmG[:, :ntl * P],
                        func=AF.Copy, accum_out=s1[:, ci:ci + 1])
                    nc.scalar.activation(
                        out=sqG[:, :ntl * P], in_=pmG[:, :ntl * P],
                        func=AF.Square, accum_out=s1q[:, ci:ci + 1])

                if stage < 5:
                    continue
                # BN1 stats allreduce
                def bn_stats(sums, sq_t, idx):
                    red = sp.tile([D, 2], F, tag="red")
                    nc.vector.tensor_reduce(out=red[:, 0:1], in_=sums[:],
                                            axis=mybir.AxisListType.X,
                                            op=AL.add)
                    nc.vector.tensor_reduce(out=red[:, 1:2], in_=sq_t[:],
                                            axis=mybir.AxisListType.X,
                                            op=AL.add)
                    nc.sync.dma_start(out=st_in[idx][:], in_=red[:])
                    nc.gpsimd.collective_compute(
                        "AllGather", AL.bypass, replica_groups=rg,
                        ins=[st_in[idx][:]], outs=[st_out[idx][:]])
                    # read back [8, 64, 2] as [64part, 8core, 2] and reduce
                    st8 = sp.tile([D, NCORES, 2], F, tag="st8")
                    full_ap = st_out[idx][:]
                    nc.sync.dma_start(
                        out=st8[:],
                        in_=bass.AP(full_ap.tensor, full_ap.offset,
                                    [[2, D], [2 * D, NCORES], [1, 2]]))
                    st = sp.tile([D, 2], F, tag="st")
                    nc.vector.tensor_reduce(out=st[:, 0:1], in_=st8[:, :, 0:1],
                                            axis=mybir.AxisListType.XY,
                                            op=AL.add)
                    nc.vector.tensor_reduce(out=st[:, 1:2], in_=st8[:, :, 1:2],
                                            axis=mybir.AxisListType.XY,
                                            op=AL.add)
                    mean = sp.tile([D, 1], F, tag="mean")
                    ex2 = sp.tile([D, 1], F, tag="ex2")
                    nc.scalar.activation(out=mean[:], in_=st[:, 0:1],
                                         func=AF.Copy, scale=1.0 / NODES)
                    nc.scalar.activation(out=ex2[:], in_=st[:, 1:2],
                                         func=AF.Copy, scale=1.0 / NODES)
                    var = sp.tile([D, 1], F, tag="var")
                    nc.vector.tensor_tensor(out=var[:], in0=mean[:],
                                            in1=mean[:], op=AL.mult)
                    nc.vector.tensor_tensor(out=var[:], in0=ex2[:],
                                            in1=var[:], op=AL.subtract)
                    nc.vector.tensor_tensor(out=var[:], in0=var[:],
                                            in1=epsc[:], op=AL.add)
                    std = sp.tile([D, 1], F, tag="std")
                    nc.scalar.activation(out=std[:], in_=var[:], func=AF.Sqrt,
                                         bias=0.0)
                    rstd = sp.tile([D, 1], F, tag="rstd")
                    nc.vector.reciprocal(rstd[:], std[:])
                    gcol = 4 * l + (0 if idx % 2 == 0 else 2)
                    scl = sp.tile([D, 1], F, tag="scl")
                    nc.vector.tensor_tensor(out=scl[:], in0=gb[:, gcol:gcol + 1],
                                            in1=rstd[:], op=AL.mult)
                    tmp = sp.tile([D, 1], F, tag="tmp")
                    nc.vector.tensor_tensor(out=tmp[:], in0=mean[:],
                                            in1=scl[:], op=AL.mult)
                    bia = sp.tile([D, 1], F, tag="bia")
                    nc.vector.tensor_tensor(out=bia[:],
                                            in0=gb[:, gcol + 1:gcol + 2],
                                            in1=tmp[:], op=AL.subtract)
                    return scl, bia

                sc1, bi1 = bn_stats(s1, s1q, 2 * l)

                # y1n = relu(BN1(y1)); y2 = W2.T @ y1n, stats
                s2 = sp.tile([D, 16], F, tag="s2")
                s2q = sp.tile([D, 16], F, tag="s2q")
                nch = (NLOC + CHUNK - 1) // CHUNK
                for ci in range(nch):
                    c0 = ci * CHUNK
                    c1 = min(c0 + CHUNK, NLOC)
                    ca = min(c1, NSH)                # apply-BN limit
                    if ca > c0:
                        nc.scalar.activation(
                            out=B2[:, c0:ca], in_=B1[:, c0:ca], func=AF.Relu,
                            bias=bi1[:], scale=sc1[:])
                    pm2 = ps_m2.tile([D, CHUNK], F, space="PSUM", tag="pm2")
                    nc.tensor.matmul(out=pm2[:, :c1 - c0], lhsT=W2t[:],
                                     rhs=B2[:, c0:c1], start=True, stop=True)
                    sq2 = sb.tile([D, CHUNK], F, tag="sq2")
                    nc.scalar.activation(
                        out=B1[:, c0:c1], in_=pm2[:, :c1 - c0], func=AF.Copy,
                        accum_out=s2[:, ci:ci + 1])
                    nc.scalar.activation(
                        out=sq2[:, :c1 - c0], in_=pm2[:, :c1 - c0],
                        func=AF.Square, accum_out=s2q[:, ci:ci + 1])

                sc2, bi2 = bn_stats(s2[:, :nch], s2q[:, :nch], 2 * l + 1)

                # h_next = relu(BN2(y2)), transpose, store / allgather.
                # Relu per 4-tile group, then re-zero the 22 pad columns.
                dst = h_out if l == NL - 1 else cc_in[l]
                for tiles2 in _CALLS:
                    ntl2, t0b = len(tiles2), tiles2[0]
                    nc.scalar.activation(
                        out=H[:, t0b * P:(t0b + ntl2) * P],
                        in_=B1[:, t0b * P:(t0b + ntl2) * P],
                        func=AF.Relu, bias=bi2[:], scale=sc2[:])
                nc.vector.memset(H[:, NSH:NLOC], 0.0)
                for tiles2 in _CALLS:
                    ntl2, t0b = len(tiles2), tiles2[0]
                    ptrG = ps_tr.tile([P, G, D], F, space="PSUM", tag="ptr")
                    for k, t in enumerate(tiles2):
                        nc.tensor.transpose(out=ptrG[:, k, :],
                                            in_=H[:, t * P:(t + 1) * P],
                                            identity=ident[:])
                    stgG = sb.tile([P, G, D], F, tag="stg")
                    nc.scalar.activation(out=stgG[:, :ntl2, :],
                                         in_=ptrG[:, :ntl2, :], func=AF.Copy)
                    for k, t in enumerate(tiles2):
                        nc.sync.dma_start(out=dst[t * P:(t + 1) * P, :],
                                          in_=stgG[:, k, :])
                if l < NL - 1:
                    nc.gpsimd.collective_compute(
                        "AllGather", AL.bypass, replica_groups=rg,
                        ins=[cc_in[l][:]], outs=[cc_out[l][:]])
            if stage < 5:
                nc.sync.dma_start(out=h_out[0:D, :], in_=H[:, 0:D])

    nc.compile()
    return nc


def _get_nc(BLO, BHI):
    if (BLO, BHI) not in _cache:
        _cache[(BLO, BHI)] = _build(BLO, BHI)
    return _cache[(BLO, BHI)]


def kernel(x, edge_index, edge_weight, mask_teams, eps, W1, b1, g1, beta1,
           W2, b2, g2, beta2, _trace=False):
    from concourse.bass_utils import run_bass_kernel_spmd

    x = np.asarray(x, np.float32)
    eps = np.asarray(eps, np.float32)
    plan = _plan(np.asarray(edge_index), np.asarray(edge_weight), eps)
    BLO, BHI = plan["BLO"], plan["BHI"]
    table, _ = _tableize(x)
    Ws, gb = _weights(eps, np.asarray(W1), np.asarray(W2), np.asarray(g1),
                      np.asarray(beta1), np.asarray(g2), np.asarray(beta2))
    iota = np.broadcast_to(np.arange(P, dtype=np.float32), (P, P)).copy()

    in_maps = []
    for c in range(NCORES):
        pc = plan["cores"][c]
        in_maps.append({
            "x_table": table, "xT_own": table[c * NLOC:(c + 1) * NLOC].T.copy(),
            "idxlo": pc["idxlo"], "idxhi": pc["idxhi"],
            "wlo": pc["wlo"], "whi": pc["whi"], "S": pc["S"],
            "Ws": Ws, "gb": gb,
        })

    nc = _get_nc(BLO, BHI)
    res = run_bass_kernel_spmd(nc, in_maps, list(range(NCORES)), trace=_trace)
    full = np.concatenate([res.results[c]["h_out"][:NSH]
                           for c in range(NCORES)], 0)
    out = full[np.asarray(mask_teams)]
    if _trace:
        kernel._last = res
    return out



# revision 3
# speedup vs baseline: 4.7572x; 4.7572x over previous
"""GIN message-passing (CentralityChannel) on 8 trn2 NeuronCores.

Strategy (graph/data parallel per sharding hint):
  - Nodes sharded: core c owns rows [c*6250, (c+1)*6250), padded to 6272 = 49
    tiles of 128. The full node-feature table h [50176, 64] is kept bf16 and
    packed two nodes per 128-column row ([25088, 128]), replicated in every
    core's DRAM, and rebuilt each layer with a bf16 AllGather (half the wire
    bytes of fp32).
  - Edges sharded by TARGET core. Per target tile (128 nodes), edges are
    grouped by source-row parity into blocks of 128 (BEV even + BOD odd
    blocks per tile, max over cores/tiles, same for every core), zero-weight
    padded. dma_gather fetches one 256B token (= the 2-node bf16 row pair)
    per edge with int16 indices src_row>>1; the parity split makes the
    64-wide half-select a static slice per block, and one gather call covers
    a 4-tile group (blocks ordered evens-of-tiles then odds-of-tiles).
  - Per block: gathered rows are weighted into bf16 msg copies (DVE), and PE
    accumulates aggT[64f, 128n] += msg.T @ S in PSUM == segment-sum of
    w*h[src]. The one-hot S[e, n] = (tgtloc[e] == n) is static across
    layers, precomputed on the host in fp8e4 (exact for 0/1) and streamed
    from DRAM per group; fp8 rhs with bf16 lhsT runs at the 1-cycle/row PE
    rate with half the S bytes of bf16.
  - GIN combine is folded: edge weights are pre-divided by (1+eps_l) on the
    host and W1_l is pre-multiplied by (1+eps_l), so y0 = hT_own + aggT.
  - MLP runs feature-major in bf16 (weights and activations; y1/y2 stats
    accumulate from fp32 PSUM): y1T = W1'.T @ y0, batched 4 tiles (512 node
    cols) per PSUM bank. BatchNorm1d (training mode) needs global batch
    stats: per-channel sum/sumsq accumulate via ACT accum_out, then an
    AllGather of the 8 partial [64,2] blocks + on-chip reduce, then
    BN+bias+ReLU is one ACT op with per-partition scale/bias. MLP biases
    cancel inside BatchNorm and are dropped.
  - New h shard is transposed back node-major via PE and AllGathered (bf16)
    into the next layer's table. The final layer skips the AllGather; the
    host assembles shards (fp32) and applies mask_teams.
"""

import sys

sys.path.insert(0, "/opt/trn_rl_repo")
import numpy as np
import ml_dtypes

NODES, D, NL = 50000, 64, 3
NCORES = 8
NSH = NODES // NCORES            # 6250
P = 128
NT = (NSH + P - 1) // P          # 49
NLOC = NT * P                    # 6272
TB = NCORES * NLOC               # 50176
BN_EPS = 1e-5
G = 4                            # target tiles per gather call
CHUNK = 512                      # node cols per MLP2 matmul

_CALLS = [list(range(s, min(s + G, NT))) for s in range(0, NT, G)]
BF_NP = ml_dtypes.bfloat16
F8_NP = ml_dtypes.float8_e4m3


def _block_layout(BEV, BOD):
    """Group-local block order: per 4-tile group, even blocks of each tile
    (tile-major), then odd blocks of each tile. Returns, per (tile, parity,
    block), the absolute block position in the concatenated stream, plus the
    per-group starting position."""
    BPT = BEV + BOD
    pos = {}
    gstart = []
    base = 0
    for tiles in _CALLS:
        gstart.append(base)
        ntl = len(tiles)
        for ti, t in enumerate(tiles):
            for b in range(BEV):
                pos[(t, 0, b)] = base + ti * BEV + b
            for b in range(BOD):
                pos[(t, 1, b)] = base + ntl * BEV + ti * BOD + b
        base += ntl * BPT
    return pos, gstart, base


def _plan(edge_index, edge_weight, eps):
    """Host preprocessing: shard/sort/pad edges into the static block layout."""
    src = edge_index[0].astype(np.int64)
    tgt = edge_index[1].astype(np.int64)
    w = edge_weight.astype(np.float32)
    assert np.all(np.abs(1.0 + eps) > 1e-6), "eps == -1 unsupported"

    src_row = (src // NSH) * NLOC + (src % NSH)     # row in padded table
    c_tgt = tgt // NSH
    r = tgt % NSH
    tile = r // P
    lane = (r % P).astype(np.float32)               # one-hot lane value
    par = (src_row & 1).astype(np.int64)

    key = (c_tgt * NT + tile) * 2 + par
    order = np.argsort(key, kind="stable")
    counts = np.bincount(key, minlength=NCORES * NT * 2)
    starts = np.zeros_like(counts)
    starts[1:] = np.cumsum(counts)[:-1]
    q = np.arange(len(src)) - starts[key[order]]    # rank within group

    BEV = int(np.ceil(counts.reshape(-1, 2)[:, 0].max() / P))
    BOD = int(np.ceil(counts.reshape(-1, 2)[:, 1].max() / P))
    BPT = BEV + BOD
    pos_map, gstart, nblk = _block_layout(BEV, BOD)
    assert nblk == NT * BPT

    # absolute block position per (tile, parity, block)
    posarr = np.zeros((NT, 2, max(BEV, BOD)), np.int64)
    for (t, pq, b), v in pos_map.items():
        posarr[t, pq, b] = v

    so, po, co, to = src_row[order], par[order], c_tgt[order], tile[order]
    lo_, wo = lane[order], w[order]
    b = q // P
    p = q % P
    bmax = np.where(po == 0, BEV, BOD)
    assert np.all(b < bmax), "block overflow; BEV/BOD too small"
    blkpos = posarr[to, po, b]                      # absolute block position

    cores = []
    for c in range(NCORES):
        m = co == c
        tok = np.zeros(NT * BPT * P, np.int64)
        wtok = np.zeros(NT * BPT * P, np.float32)
        ttok = np.zeros(NT * BPT * P, np.float32)
        pp = blkpos[m] * P + p[m]
        tok[pp] = so[m] >> 1
        wtok[pp] = wo[m]
        ttok[pp] = lo_[m]

        # int16 index stream, 16-partition wrap, per gather call
        outs = []
        for gi, tiles in enumerate(_CALLS):
            nb = len(tiles) * BPT
            seg = tok[gstart[gi] * P:(gstart[gi] + nb) * P]
            outs.append(seg.reshape(-1, 16).T)
        idx = np.tile(np.concatenate(outs, axis=1).astype(np.int16), (8, 1))

        wl = np.stack([wtok / (1.0 + eps[l]) for l in range(NL)])
        wt = np.concatenate(
            [a.reshape(NT * BPT, P).T for a in wl], 1).astype(BF_NP)
        lanes = ttok.reshape(NT * BPT, P)           # [blk, p] lane values
        one_hot = (lanes[:, :, None] ==
                   np.arange(P, dtype=np.float32)[None, None, :])
        S8 = np.ascontiguousarray(
            one_hot.transpose(1, 0, 2).reshape(P, NT * BPT * P)
        ).astype(F8_NP)                             # [p, blk*n]
        cores.append(dict(idx=idx, wt=wt, S=S8,
                          tok=tok, wtok=wtok, ttokf=ttok))
    return dict(BEV=BEV, BOD=BOD, BPT=BPT, gstart=gstart, cores=cores)


def _tableize(x):
    rows = (np.arange(NODES) // NSH) * NLOC + np.arange(NODES) % NSH
    tb = np.zeros((TB, D), np.float32)
    tb[rows] = x
    return tb, rows


def _weights(eps, W1, W2, g1, beta1, g2, beta2):
    ws = []
    for l in range(NL):
        ws.append(((1.0 + eps[l]) * W1[l]).astype(np.float32))
        ws.append(W2[l].astype(np.float32))
    Ws = np.concatenate(ws, 0)                       # [NL*2*64, 64]
    gb = np.stack(sum([[g1[l], beta1[l], g2[l], beta2[l]] for l in range(NL)],
                      []), 1).astype(np.float32)     # [64, 12]
    return Ws, gb


def mirror(x, edge_index, edge_weight, mask_teams, eps, W1, b1, g1, beta1,
           W2, b2, g2, beta2):
    """Numpy mirror of the device computation (for validation)."""
    plan = _plan(np.asarray(edge_index), np.asarray(edge_weight),
                 np.asarray(eps))
    BPT = plan["BPT"]
    table, rows = _tableize(np.asarray(x))
    Ws, gb = _weights(eps, W1, W2, g1, beta1, g2, beta2)
    Wsb = Ws.astype(BF_NP).astype(np.float32)
    H = [table[c * NLOC:(c + 1) * NLOC].T.copy() for c in range(NCORES)]
    tbl = table.astype(BF_NP).astype(np.float32)
    pair = tbl.reshape(TB // 2, 2 * D)

    for l in range(NL):
        aggs = []
        for c in range(NCORES):
            pc = plan["cores"][c]
            wq = (pc["wtok"] / (1 + eps[l])).astype(BF_NP).astype(np.float32)
            # device gathers the 2-node token; the parity-static slice picks
            # the right half. tok holds src_row>>1; parity known per block
            # position (evens first within group) — emulate via src parity
            # stored implicitly: reconstruct from ttok/wtok is not possible,
            # so emulate exactly: gather both halves and pick by parity of
            # the original source row. We stored tok = src_row>>1 only, so
            # recompute parity from the plan inputs is needed; instead use
            # the fact that pad slots have w=0 and real slots follow block
            # parity.
            BEV = plan["BEV"]
            nblk = NT * BPT
            tokens = pair[pc["tok"]]                # [slots, 128]
            blk = np.arange(nblk * P) // P
            # parity per absolute block position
            parv = np.zeros(nblk, np.int64)
            posm, gstart, _ = _block_layout(plan["BEV"], plan["BOD"])
            for (t, pq, b), v in posm.items():
                parv[v] = pq
            off = parv[blk] * D
            msg = np.take_along_axis(
                tokens, off[:, None] + np.arange(D)[None, :], axis=1)
            msg = (msg.astype(BF_NP).astype(np.float32) *
                   wq[:, None]).reshape(nblk, P, D)
            tl = pc["ttokf"].reshape(nblk, P)
            S = (tl[..., None] == np.arange(P, dtype=np.float32)).astype(
                np.float32)                           # [nblk, P, Pn]
            # map absolute block position back to tile
            tilev = np.zeros(nblk, np.int64)
            for (t, pq, b), v in posm.items():
                tilev[v] = t
            agg = np.zeros((D, NLOC), np.float32)
            part = np.einsum("bpd,bpn->bdn", msg, S)
            for bi in range(nblk):
                t = tilev[bi]
                agg[:, t * P:(t + 1) * P] += part[bi]
            aggs.append(agg)
        y1s = []
        for c in range(NCORES):
            y0 = (H[c] + aggs[c]).astype(BF_NP).astype(np.float32)
            y1s.append(Wsb[2 * l * D:(2 * l + 1) * D].T @ y0)
        s1 = sum(y[:, :NSH].sum(1) for y in y1s)
        s1q = sum((y[:, :NSH] ** 2).sum(1) for y in y1s)
        mu, ex2 = s1 / NODES, s1q / NODES
        sc1 = gb[:, 4 * l + 0] / np.sqrt(ex2 - mu ** 2 + BN_EPS)
        bi1 = gb[:, 4 * l + 1] - mu * sc1
        y2s = []
        for c in range(NCORES):
            y1n = np.zeros_like(y1s[c])
            y1n[:, :NSH] = np.maximum(
                y1s[c][:, :NSH] * sc1[:, None] + bi1[:, None], 0)
            y1n = y1n.astype(BF_NP).astype(np.float32)
            y2s.append(Wsb[(2 * l + 1) * D:(2 * l + 2) * D].T @ y1n)
        s2 = sum(y[:, :NSH].sum(1) for y in y2s)
        s2q = sum((y[:, :NSH] ** 2).sum(1) for y in y2s)
        mu2, ex22 = s2 / NODES, s2q / NODES
        sc2 = gb[:, 4 * l + 2] / np.sqrt(ex22 - mu2 ** 2 + BN_EPS)
        bi2 = gb[:, 4 * l + 3] - mu2 * sc2
        for c in range(NCORES):
            hn = np.zeros_like(y2s[c])
            hn[:, :NSH] = np.maximum(
                y2s[c][:, :NSH] * sc2[:, None] + bi2[:, None], 0)
            H[c] = hn
            table[c * NLOC:(c + 1) * NLOC] = hn.T
        tbl = table.astype(BF_NP).astype(np.float32)
        pair = tbl.reshape(TB // 2, 2 * D)
    full = np.concatenate([H[c].T[:NSH] for c in range(NCORES)], 0)
    return full[np.asarray(mask_teams)]


# ---------------------------------------------------------------------------
# Device program
# ---------------------------------------------------------------------------
_cache = {}


def _build(BEV, BOD, stage=5):
    from concourse import bass, bacc, mybir, tile
    from concourse.masks import make_identity

    F = mybir.dt.float32
    BF = mybir.dt.bfloat16
    F8 = mybir.dt.float8e4
    I16 = mybir.dt.int16
    BPT = BEV + BOD
    AL = mybir.AluOpType
    AF = mybir.ActivationFunctionType

    nc = bacc.Bacc(num_devices=NCORES, num_swdge_queues=2)
    x_table = nc.declare_dram_parameter("x_table", [TB // 2, 2 * D], BF, False)
    xT_own = nc.declare_dram_parameter("xT_own", [D, NLOC], F, isOutput=False)
    idx_in = nc.declare_dram_parameter("idx", [P, NT * BPT * 8], I16, False)
    wt_in = nc.declare_dram_parameter("wt", [P, NL * NT * BPT], BF, False)
    S_in = nc.declare_dram_parameter("S", [P, NT * BPT * P], F8, False)
    Ws_in = nc.declare_dram_parameter("Ws", [NL * 2 * D, D], BF, False)
    gb_in = nc.declare_dram_parameter("gb", [D, 4 * NL], F, False)
    h_out = nc.declare_dram_parameter("h_out", [NLOC, D], F, isOutput=True)

    cc_in = [nc.dram_tensor(f"cc_in{l}", [NLOC, D], BF) for l in range(NL - 1)]
    cc_out = [nc.dram_tensor(f"cc_out{l}", [TB, D], BF, addr_space="Shared")
              for l in range(NL - 1)]
    st_in = [nc.dram_tensor(f"st_in{i}", [D, 2], F) for i in range(2 * NL)]
    st_out = [nc.dram_tensor(f"st_out{i}", [NCORES * D, 2], F,
                             addr_space="Shared") for i in range(2 * NL)]
    rg = [list(range(NCORES))]

    with tile.TileContext(nc) as tc:
        with (
            tc.tile_pool(name="persist", bufs=1) as pp,
            tc.tile_pool(name="gat", bufs=2) as gp,
            tc.tile_pool(name="sg", bufs=2) as sgp,
            tc.tile_pool(name="sb", bufs=2) as sb,
            tc.tile_pool(name="small", bufs=4) as sp,
            tc.tile_pool(name="ps_agg", bufs=2, space="PSUM") as ps_agg,
            tc.tile_pool(name="ps_m", bufs=2, space="PSUM") as ps_m,
            tc.tile_pool(name="ps_m2", bufs=2, space="PSUM") as ps_m2,
            tc.tile_pool(name="ps_tr", bufs=2, space="PSUM") as ps_tr,
        ):
            # resident tiles
            H = pp.tile([D, NLOC], F)
            B1 = pp.tile([D, NLOC], F)
            B2 = pp.tile([D, NLOC], BF)
            idxt = pp.tile([P, NT * BPT * 8], I16)
            wt = pp.tile([P, NL * NT * BPT], BF)
            gb = pp.tile([D, 4 * NL], F)
            ident = pp.tile([D, D], F)

            nc.sync.dma_start(out=H[:], in_=xT_own[:])
            nc.sync.dma_start(out=idxt[:], in_=idx_in[:])
            nc.sync.dma_start(out=wt[:], in_=wt_in[:])
            nc.sync.dma_start(out=gb[:], in_=gb_in[:])
            make_identity(nc, ident[:])
            nc.vector.memset(B1[:], 0.0)
            nc.vector.memset(B2[:], 0.0)
            epsc = pp.tile([D, 1], F)
            nc.vector.memset(epsc[:], BN_EPS)

            for l in range(NL):
                if l == 0:
                    tab_ap = x_table[:]
                else:
                    t_ = cc_out[l - 1][:]
                    tab_ap = bass.AP(t_.tensor, t_.offset,
                                     [[2 * D, TB // 2], [1, 2 * D]])
                W1t = sp.tile([D, D], BF, tag="w1")
                W2t = sp.tile([D, D], BF, tag="w2")
                nc.sync.dma_start(out=W1t[:], in_=Ws_in[2 * l * D:(2 * l + 1) * D, :])
                nc.sync.dma_start(out=W2t[:], in_=Ws_in[(2 * l + 1) * D:(2 * l + 2) * D, :])
                NG = len(_CALLS)
                s1 = sp.tile([D, NG], F, tag="s1")
                s1q = sp.tile([D, NG], F, tag="s1q")

                gpos = 0                              # running block position
                for ci, tiles in enumerate(_CALLS):
                    ntl = len(tiles)
                    t0 = tiles[0]
                    nb = ntl * BPT
                    g0 = gpos
                    gpos += nb
                    nev = ntl * BEV
                    glo = gp.tile([P, G * BPT, 2 * D], BF, tag="glo")
                    if stage < 1:
                        continue
                    Sg = sgp.tile([P, G * BPT, P], F8, tag="Sg")
                    nc.sync.dma_start(
                        out=Sg[:, :nb, :],
                        in_=S_in[:, g0 * P:(g0 + nb) * P])
                    nc.gpsimd.dma_gather(
                        out_ap=glo[:, :nb, :], in_ap=tab_ap,
                        idxs_ap=idxt[:, g0 * 8:(g0 + nb) * 8],
                        num_idxs=nb * P, num_idxs_reg=nb * P,
                        elem_size=2 * D, single_packet=False)
                    if stage < 2:
                        continue
                    # weight messages: evens read token half [0:64], odds
                    # read [64:128]; block order per group is evens-of-tiles
                    # then odds-of-tiles, so both are static slices.
                    glob = gp.tile([P, G * BPT, D], BF, tag="glob")
                    wrow = (l * NT) * BPT + g0
                    nc.vector.tensor_tensor(
                        out=glob[:, :nev, :],
                        in0=glo[:, :nev, 0:D],
                        in1=wt[:, wrow:wrow + nev].to_broadcast([P, nev, D]),
                        op=AL.mult)
                    nc.vector.tensor_tensor(
                        out=glob[:, nev:nb, :],
                        in0=glo[:, nev:nb, D:2 * D],
                        in1=wt[:, wrow + nev:wrow + nb]
                        .to_broadcast([P, nb - nev, D]),
                        op=AL.mult)

                    if stage < 3:
                        continue
                    if stage < 4:
                        continue
                    paG = ps_agg.tile([D, G * P], F, space="PSUM", tag="pa")
                    for ti, t in enumerate(tiles):
                        blocks = [ti * BEV + b for b in range(BEV)] + \
                                 [nev + ti * BOD + b for b in range(BOD)]
                        for k, pb in enumerate(blocks):
                            nc.tensor.matmul(
                                out=paG[:, ti * P:(ti + 1) * P],
                                lhsT=glob[:, pb, :],
                                rhs=Sg[:, pb, :],
                                start=(k == 0), stop=(k == BPT - 1))
                    # combine + MLP1, one shot per 4-tile group (pads are
                    # exactly zero in H and agg, so stats over them are safe)
                    y0b = sb.tile([D, G * P], BF, tag="y0")
                    nc.vector.tensor_tensor(
                        out=y0b[:, :ntl * P], in0=paG[:, :ntl * P],
                        in1=H[:, t0 * P:(t0 + ntl) * P], op=AL.add)
                    pmG = ps_m.tile([D, G * P], F, space="PSUM", tag="pm")
                    nc.tensor.matmul(out=pmG[:, :ntl * P], lhsT=W1t[:],
                                     rhs=y0b[:, :ntl * P],
                                     start=True, stop=True)
                    sqG = sb.tile([D, G * P], F, tag="sq")
                    nc.scalar.activation(
                        out=B1[:, t0 * P:(t0 + ntl) * P],
                        in_=pmG[:, :ntl * P],
                        func=AF.Copy, accum_out=s1[:, ci:ci + 1])
                    nc.scalar.activation(
                        out=sqG[:, :ntl * P], in_=pmG[:, :ntl * P],
                        func=AF.Square, accum_out=s1q[:, ci:ci + 1])

                if stage < 5:
                    continue
                # BN stats allreduce (AllGather of [64,2] partials + reduce)
                def bn_stats(sums, sq_t, idx):
                    red = sp.tile([D, 2], F, tag="red")
                    nc.vector.tensor_reduce(out=red[:, 0:1], in_=sums[:],
                                            axis=mybir.AxisListType.X,
                                            op=AL.add)
                    nc.vector.tensor_reduce(out=red[:, 1:2], in_=sq_t[:],
                                            axis=mybir.AxisListType.X,
                                            op=AL.add)
                    nc.sync.dma_start(out=st_in[idx][:], in_=red[:])
                    nc.gpsimd.collective_compute(
                        "AllGather", AL.bypass, replica_groups=rg,
                        ins=[st_in[idx][:]], outs=[st_out[idx][:]])
                    st8 = sp.tile([D, NCORES, 2], F, tag="st8")
                    full_ap = st_out[idx][:]
                    nc.sync.dma_start(
                        out=st8[:],
                        in_=bass.AP(full_ap.tensor, full_ap.offset,
                                    [[2, D], [2 * D, NCORES], [1, 2]]))
                    st = sp.tile([D, 2], F, tag="st")
                    nc.vector.tensor_reduce(out=st[:, 0:1], in_=st8[:, :, 0:1],
                                            axis=mybir.AxisListType.XY,
                                            op=AL.add)
                    nc.vector.tensor_reduce(out=st[:, 1:2], in_=st8[:, :, 1:2],
                                            axis=mybir.AxisListType.XY,
                                            op=AL.add)
                    mean = sp.tile([D, 1], F, tag="mean")
                    ex2 = sp.tile([D, 1], F, tag="ex2")
                    nc.scalar.activation(out=mean[:], in_=st[:, 0:1],
                                         func=AF.Copy, scale=1.0 / NODES)
                    nc.scalar.activation(out=ex2[:], in_=st[:, 1:2],
                                         func=AF.Copy, scale=1.0 / NODES)
                    var = sp.tile([D, 1], F, tag="var")
                    nc.vector.tensor_tensor(out=var[:], in0=mean[:],
                                            in1=mean[:], op=AL.mult)
                    nc.vector.tensor_tensor(out=var[:], in0=ex2[:],
                                            in1=var[:], op=AL.subtract)
                    nc.vector.tensor_tensor(out=var[:], in0=var[:],
                                            in1=epsc[:], op=AL.add)
                    std = sp.tile([D, 1], F, tag="std")
                    nc.scalar.activation(out=std[:], in_=var[:], func=AF.Sqrt,
                                         bias=0.0)
                    rstd = sp.tile([D, 1], F, tag="rstd")
                    nc.vector.reciprocal(rstd[:], std[:])
                    gcol = 4 * l + (0 if idx % 2 == 0 else 2)
                    scl = sp.tile([D, 1], F, tag="scl")
                    nc.vector.tensor_tensor(out=scl[:], in0=gb[:, gcol:gcol + 1],
                                            in1=rstd[:], op=AL.mult)
                    tmp = sp.tile([D, 1], F, tag="tmp")
                    nc.vector.tensor_tensor(out=tmp[:], in0=mean[:],
                                            in1=scl[:], op=AL.mult)
                    bia = sp.tile([D, 1], F, tag="bia")
                    nc.vector.tensor_tensor(out=bia[:],
                                            in0=gb[:, gcol + 1:gcol + 2],
                                            in1=tmp[:], op=AL.subtract)
                    return scl, bia

                sc1, bi1 = bn_stats(s1, s1q, 2 * l)

                # y1n = relu(BN1(y1)) in bf16; y2 = W2.T @ y1n, stats
                s2 = sp.tile([D, 16], F, tag="s2")
                s2q = sp.tile([D, 16], F, tag="s2q")
                nch = (NLOC + CHUNK - 1) // CHUNK
                for ci in range(nch):
                    c0 = ci * CHUNK
                    c1 = min(c0 + CHUNK, NLOC)
                    ca = min(c1, NSH)                # apply-BN limit
                    if ca > c0:
                        nc.scalar.activation(
                            out=B2[:, c0:ca], in_=B1[:, c0:ca], func=AF.Relu,
                            bias=bi1[:], scale=sc1[:])
                    pm2 = ps_m2.tile([D, CHUNK], F, space="PSUM", tag="pm2")
                    nc.tensor.matmul(out=pm2[:, :c1 - c0], lhsT=W2t[:],
                                     rhs=B2[:, c0:c1], start=True, stop=True)
                    sq2 = sb.tile([D, CHUNK], F, tag="sq2")
                    nc.scalar.activation(
                        out=B1[:, c0:c1], in_=pm2[:, :c1 - c0], func=AF.Copy,
                        accum_out=s2[:, ci:ci + 1])
                    nc.scalar.activation(
                        out=sq2[:, :c1 - c0], in_=pm2[:, :c1 - c0],
                        func=AF.Square, accum_out=s2q[:, ci:ci + 1])

                sc2, bi2 = bn_stats(s2[:, :nch], s2q[:, :nch], 2 * l + 1)

                # h_next = relu(BN2(y2)), transpose, store / allgather.
                dst = h_out if l == NL - 1 else cc_in[l]
                for tiles2 in _CALLS:
                    ntl2, t0b = len(tiles2), tiles2[0]
                    nc.scalar.activation(
                        out=H[:, t0b * P:(t0b + ntl2) * P],
                        in_=B1[:, t0b * P:(t0b + ntl2) * P],
                        func=AF.Relu, bias=bi2[:], scale=sc2[:])
                nc.vector.memset(H[:, NSH:NLOC], 0.0)
                for tiles2 in _CALLS:
                    ntl2, t0b = len(tiles2), tiles2[0]
                    ptrG = ps_tr.tile([P, G, D], F, space="PSUM", tag="ptr")
                    for k, t in enumerate(tiles2):
                        nc.tensor.transpose(out=ptrG[:, k, :],
                                            in_=H[:, t * P:(t + 1) * P],
                                            identity=ident[:])
                    if l == NL - 1:
                        stgG = sb.tile([P, G, D], F, tag="stgf")
                    else:
                        stgG = sb.tile([P, G, D], BF, tag="stgb")
                    nc.scalar.activation(out=stgG[:, :ntl2, :],
                                         in_=ptrG[:, :ntl2, :], func=AF.Copy)
                    for k, t in enumerate(tiles2):
                        nc.sync.dma_start(out=dst[t * P:(t + 1) * P, :],
                                          in_=stgG[:, k, :])
                if l < NL - 1:
                    nc.gpsimd.collective_compute(
                        "AllGather", AL.bypass, replica_groups=rg,
                        ins=[cc_in[l][:]], outs=[cc_out[l][:]])
            if stage < 5:
                nc.sync.dma_start(out=h_out[0:D, :], in_=H[:, 0:D])

    nc.compile()
    return nc


def _get_nc(BEV, BOD):
    if (BEV, BOD) not in _cache:
        _cache[(BEV, BOD)] = _build(BEV, BOD)
    return _cache[(BEV, BOD)]


def _in_maps(plan, table, Ws, gb):
    maps = []
    tbf = table.astype(BF_NP).reshape(TB // 2, 2 * D)
    Wsb = Ws.astype(BF_NP)
    for c in range(NCORES):
        pc = plan["cores"][c]
        maps.append({
            "x_table": tbf,
            "xT_own": table[c * NLOC:(c + 1) * NLOC].T.copy(),
            "idx": pc["idx"], "wt": pc["wt"], "S": pc["S"],
            "Ws": Wsb, "gb": gb,
        })
    return maps


def kernel(x, edge_index, edge_weight, mask_teams, eps, W1, b1, g1, beta1,
           W2, b2, g2, beta2, _trace=False):
    from concourse.bass_utils import run_bass_kernel_spmd

    x = np.asarray(x, np.float32)
    eps = np.asarray(eps, np.float32)
    plan = _plan(np.asarray(edge_index), np.asarray(edge_weight), eps)
    table, _ = _tableize(x)
    Ws, gb = _weights(eps, np.asarray(W1), np.asarray(W2), np.asarray(g1),
                      np.asarray(beta1), np.asarray(g2), np.asarray(beta2))
    in_maps = _in_maps(plan, table, Ws, gb)

    nc = _get_nc(plan["BEV"], plan["BOD"])
    res = run_bass_kernel_spmd(nc, in_maps, list(range(NCORES)), trace=_trace)
    full = np.concatenate([res.results[c]["h_out"][:NSH]
                           for c in range(NCORES)], 0)
    out = full[np.asarray(mask_teams)]
    if _trace:
        kernel._last = res
    return out


# revision 25
# speedup vs baseline: 32.1549x; 6.7592x over previous
"""GIN message-passing (CentralityChannel) on 8 trn2 NeuronCores.

Strategy (graph/data parallel per sharding hint):
  - Nodes sharded: core c owns rows [c*6250, (c+1)*6250), padded to 6272 = 49
    tiles of 128. The full node-feature table h [50176, 64] is kept fp16 and
    packed two nodes per 128-column row ([25088, 128]), replicated in every
    core's DRAM, and rebuilt each layer with an fp16 AllGather (half the wire
    bytes of fp32).
  - Edges sharded by TARGET core. Per target tile (128 nodes), edges are
    grouped by source-row parity into blocks of 128 (BEV even + BOD odd
    blocks per tile, max over cores/tiles, same for every core), zero-weight
    padded. dma_gather fetches one 256B token (= the 2-node fp16 row pair)
    per edge with int16 indices src_row>>1; the parity split makes the
    64-wide half-select a static slice per block, and one gather call covers
    a 4-tile group (blocks ordered evens-of-tiles then odds-of-tiles).
  - Per block: gathered rows are weighted in place (DVE), and PE accumulates
    aggT[64f, 128n] += msg.T @ S in PSUM == segment-sum of w*h[src]. The
    one-hot S[e, n] = (tgtloc[e] == n) is static across layers, precomputed
    on the host in fp8e4 (exact for 0/1) and streamed from DRAM per group on
    the ACT HWDGE queue (prefetches into collective stall windows); fp8 rhs
    with fp16 lhsT runs the agg matmuls at the 1-cycle/row PE rate with half
    the S bytes of bf16.
  - GIN combine is folded: edge weights are pre-divided by (1+eps_l) on the
    host and W1_l is pre-multiplied by (1+eps_l), so y0 = hT_own + aggT.
  - MLP runs feature-major in fp16 (weights and activations; y1/y2 stats
    accumulate from fp32 PSUM; sum-of-squares reduces on DVE since ACT is
    the MLP2-phase bottleneck): y1T = W1'.T @ y0, batched 4 tiles (512 node
    cols) per PSUM bank. BatchNorm1d (training mode) needs global batch
    stats: per-channel sum/sumsq accumulate via ACT accum_out, then an
    AllGather of the 8 partial [64,2] blocks + on-chip reduce, then
    BN+bias+ReLU is one ACT op with per-partition scale/bias. MLP biases
    cancel inside BatchNorm and are dropped.
  - New h shard is transposed back node-major via PE into a persistent
    staging tile and stored with ONE DMA per layer (49 small stores would be
    HWDGE-fixed-cost dominated), then AllGathered (fp16) into the next
    layer's table. The final layer ships raw y2 + local BN stats; the host
    reduces stats and applies BN2+ReLU to just the masked rows (saves one
    collective round-trip).
"""

import sys

sys.path.insert(0, "/opt/trn_rl_repo")
import numpy as np
import ml_dtypes

NODES, D, NL = 50000, 64, 3
NCORES = 8
NSH = NODES // NCORES            # 6250
P = 128
NT = (NSH + P - 1) // P          # 49
NLOC = NT * P                    # 6272
TB = NCORES * NLOC               # 50176
BN_EPS = 1e-5
G = 4                            # target tiles per gather call
CHUNK = 512                      # node cols per MLP2 matmul

_CALLS = [list(range(s, min(s + G, NT))) for s in range(0, NT, G)]
H_NP = ml_dtypes.bfloat16        # 2-byte activation dtype (bf16: fp16 x fp8
F8_NP = ml_dtypes.float8_e4m3    # matmul pairing hangs the PE on this HW)


def _block_layout(BEV, BOD):
    """Group-local block order: per 4-tile group, even blocks of each tile
    (tile-major), then odd blocks of each tile. Returns, per (tile, parity,
    block), the absolute block position in the concatenated stream, plus the
    per-group starting position."""
    BPT = BEV + BOD
    pos = {}
    gstart = []
    base = 0
    for tiles in _CALLS:
        gstart.append(base)
        ntl = len(tiles)
        for ti, t in enumerate(tiles):
            for b in range(BEV):
                pos[(t, 0, b)] = base + ti * BEV + b
            for b in range(BOD):
                pos[(t, 1, b)] = base + ntl * BEV + ti * BOD + b
        base += ntl * BPT
    return pos, gstart, base


def _plan(edge_index, edge_weight, eps):
    """Host preprocessing: shard/sort/pad edges into the static block layout."""
    src = edge_index[0].astype(np.int64)
    tgt = edge_index[1].astype(np.int64)
    w = edge_weight.astype(np.float32)
    assert np.all(np.abs(1.0 + eps) > 1e-6), "eps == -1 unsupported"

    src_row = (src // NSH) * NLOC + (src % NSH)     # row in padded table
    c_tgt = tgt // NSH
    r = tgt % NSH
    tile = r // P
    lane = (r % P).astype(np.float32)               # one-hot lane value
    par = (src_row & 1).astype(np.int64)

    key = (c_tgt * NT + tile) * 2 + par
    order = np.argsort(key, kind="stable")
    counts = np.bincount(key, minlength=NCORES * NT * 2)
    starts = np.zeros_like(counts)
    starts[1:] = np.cumsum(counts)[:-1]
    q = np.arange(len(src)) - starts[key[order]]    # rank within group

    BEV = int(np.ceil(counts.reshape(-1, 2)[:, 0].max() / P))
    BOD = int(np.ceil(counts.reshape(-1, 2)[:, 1].max() / P))
    BPT = BEV + BOD
    pos_map, gstart, nblk = _block_layout(BEV, BOD)
    assert nblk == NT * BPT

    # absolute block position per (tile, parity, block)
    posarr = np.zeros((NT, 2, max(BEV, BOD)), np.int64)
    for (t, pq, b), v in pos_map.items():
        posarr[t, pq, b] = v

    so, po, co, to = src_row[order], par[order], c_tgt[order], tile[order]
    lo_, wo = lane[order], w[order]
    b = q // P
    p = q % P
    bmax = np.where(po == 0, BEV, BOD)
    assert np.all(b < bmax), "block overflow; BEV/BOD too small"
    blkpos = posarr[to, po, b]                      # absolute block position

    cores = []
    for c in range(NCORES):
        m = co == c
        tok = np.zeros(NT * BPT * P, np.int64)
        wtok = np.zeros(NT * BPT * P, np.float32)
        ttok = np.zeros(NT * BPT * P, np.float32)
        pp = blkpos[m] * P + p[m]
        tok[pp] = so[m] >> 1
        wtok[pp] = wo[m]
        ttok[pp] = lo_[m]

        # int16 index stream, 16-partition wrap, per gather call
        outs = []
        for gi, tiles in enumerate(_CALLS):
            nb = len(tiles) * BPT
            seg = tok[gstart[gi] * P:(gstart[gi] + nb) * P]
            outs.append(seg.reshape(-1, 16).T)
        idx = np.tile(np.concatenate(outs, axis=1).astype(np.int16), (8, 1))

        wl = np.stack([wtok / (1.0 + eps[l]) for l in range(NL)])
        wt = np.concatenate(
            [a.reshape(NT * BPT, P).T for a in wl], 1).astype(H_NP)
        lanes = ttok.reshape(NT * BPT, P)           # [blk, p] lane values
        one_hot = (lanes[:, :, None] ==
                   np.arange(P, dtype=np.float32)[None, None, :])
        S8 = np.ascontiguousarray(
            one_hot.transpose(1, 0, 2).reshape(P, NT * BPT * P)
        ).astype(F8_NP)                             # [p, blk*n]
        cores.append(dict(idx=idx, wt=wt, S=S8,
                          tok=tok, wtok=wtok, ttokf=ttok))
    return dict(BEV=BEV, BOD=BOD, BPT=BPT, gstart=gstart, cores=cores)


def _tableize(x):
    rows = (np.arange(NODES) // NSH) * NLOC + np.arange(NODES) % NSH
    tb = np.zeros((TB, D), np.float32)
    tb[rows] = x
    return tb, rows


def _weights(eps, W1, W2, g1, beta1, g2, beta2):
    ws = []
    for l in range(NL):
        ws.append(((1.0 + eps[l]) * W1[l]).astype(np.float32))
        ws.append(W2[l].astype(np.float32))
    Ws = np.concatenate(ws, 0)                       # [NL*2*64, 64]
    gb = np.stack(sum([[g1[l], beta1[l], g2[l], beta2[l]] for l in range(NL)],
                      []), 1).astype(np.float32)     # [64, 12]
    return Ws, gb


def mirror(x, edge_index, edge_weight, mask_teams, eps, W1, b1, g1, beta1,
           W2, b2, g2, beta2):
    """Numpy mirror of the device computation (for validation)."""
    plan = _plan(np.asarray(edge_index), np.asarray(edge_weight),
                 np.asarray(eps))
    BPT = plan["BPT"]
    table, rows = _tableize(np.asarray(x))
    Ws, gb = _weights(eps, W1, W2, g1, beta1, g2, beta2)
    Wsb = Ws.astype(H_NP).astype(np.float32)
    H = [table[c * NLOC:(c + 1) * NLOC].T.copy() for c in range(NCORES)]
    tbl = table.astype(H_NP).astype(np.float32)
    pair = tbl.reshape(TB // 2, 2 * D)

    for l in range(NL):
        aggs = []
        for c in range(NCORES):
            pc = plan["cores"][c]
            wq = (pc["wtok"] / (1 + eps[l])).astype(H_NP).astype(np.float32)
            # device gathers the 2-node token; the parity-static slice picks
            # the right half. tok holds src_row>>1; parity known per block
            # position (evens first within group) — emulate via src parity
            # stored implicitly: reconstruct from ttok/wtok is not possible,
            # so emulate exactly: gather both halves and pick by parity of
            # the original source row. We stored tok = src_row>>1 only, so
            # recompute parity from the plan inputs is needed; instead use
            # the fact that pad slots have w=0 and real slots follow block
            # parity.
            BEV = plan["BEV"]
            nblk = NT * BPT
            tokens = pair[pc["tok"]]                # [slots, 128]
            blk = np.arange(nblk * P) // P
            # parity per absolute block position
            parv = np.zeros(nblk, np.int64)
            posm, gstart, _ = _block_layout(plan["BEV"], plan["BOD"])
            for (t, pq, b), v in posm.items():
                parv[v] = pq
            off = parv[blk] * D
            msg = np.take_along_axis(
                tokens, off[:, None] + np.arange(D)[None, :], axis=1)
            msg = (msg.astype(H_NP).astype(np.float32) *
                   wq[:, None]).reshape(nblk, P, D)
            tl = pc["ttokf"].reshape(nblk, P)
            S = (tl[..., None] == np.arange(P, dtype=np.float32)).astype(
                np.float32)                           # [nblk, P, Pn]
            # map absolute block position back to tile
            tilev = np.zeros(nblk, np.int64)
            for (t, pq, b), v in posm.items():
                tilev[v] = t
            agg = np.zeros((D, NLOC), np.float32)
            part = np.einsum("bpd,bpn->bdn", msg, S)
            for bi in range(nblk):
                t = tilev[bi]
                agg[:, t * P:(t + 1) * P] += part[bi]
            aggs.append(agg)
        y1s = []
        for c in range(NCORES):
            y0 = (H[c] + aggs[c]).astype(H_NP).astype(np.float32)
            y1s.append(Wsb[2 * l * D:(2 * l + 1) * D].T @ y0)
        s1 = sum(y[:, :NSH].sum(1) for y in y1s)
        s1q = sum((y[:, :NSH] ** 2).sum(1) for y in y1s)
        mu, ex2 = s1 / NODES, s1q / NODES
        sc1 = gb[:, 4 * l + 0] / np.sqrt(ex2 - mu ** 2 + BN_EPS)
        bi1 = gb[:, 4 * l + 1] - mu * sc1
        y2s = []
        for c in range(NCORES):
            y1n = np.zeros_like(y1s[c])
            y1n[:, :NSH] = np.maximum(
                y1s[c][:, :NSH] * sc1[:, None] + bi1[:, None], 0)
            y1n = y1n.astype(H_NP).astype(np.float32)
            y2s.append(Wsb[(2 * l + 1) * D:(2 * l + 2) * D].T @ y1n)
        s2 = sum(y[:, :NSH].sum(1) for y in y2s)
        s2q = sum((y[:, :NSH] ** 2).sum(1) for y in y2s)
        mu2, ex22 = s2 / NODES, s2q / NODES
        sc2 = gb[:, 4 * l + 2] / np.sqrt(ex22 - mu2 ** 2 + BN_EPS)
        bi2 = gb[:, 4 * l + 3] - mu2 * sc2
        for c in range(NCORES):
            hn = np.zeros_like(y2s[c])
            hn[:, :NSH] = np.maximum(
                y2s[c][:, :NSH] * sc2[:, None] + bi2[:, None], 0)
            H[c] = hn
            table[c * NLOC:(c + 1) * NLOC] = hn.T
        tbl = table.astype(H_NP).astype(np.float32)
        pair = tbl.reshape(TB // 2, 2 * D)
    full = np.concatenate([H[c].T[:NSH] for c in range(NCORES)], 0)
    return full[np.asarray(mask_teams)]


# ---------------------------------------------------------------------------
# Device program
# ---------------------------------------------------------------------------
_cache = {}


def _build(BEV, BOD, stage=5):
    from concourse import bass, bacc, mybir, tile
    from concourse.masks import make_identity

    F = mybir.dt.float32
    F16 = mybir.dt.bfloat16      # see H_NP note: stay on bf16 with fp8 rhs
    F8 = mybir.dt.float8e4
    I16 = mybir.dt.int16
    BPT = BEV + BOD
    AL = mybir.AluOpType
    AF = mybir.ActivationFunctionType

    nc = bacc.Bacc(num_devices=NCORES, num_swdge_queues=2)
    x_table = nc.declare_dram_parameter("x_table", [TB // 2, 2 * D], F16, False)
    xT_own = nc.declare_dram_parameter("xT_own", [D, NLOC], F, isOutput=False)
    idx_in = nc.declare_dram_parameter("idx", [P, NT * BPT * 8], I16, False)
    wt_in = nc.declare_dram_parameter("wt", [P, NL * NT * BPT], F16, False)
    S_in = nc.declare_dram_parameter("S", [P, NT * BPT * P], F8, False)
    Ws_in = nc.declare_dram_parameter("Ws", [NL * 2 * D, D], F16, False)
    gb_in = nc.declare_dram_parameter("gb", [D, 4 * NL], F, False)
    h_out = nc.declare_dram_parameter("h_out", [NLOC, D], F, isOutput=True)
    st2_out = nc.declare_dram_parameter("st2", [D, 2], F, isOutput=True)

    cc_in = [nc.dram_tensor(f"cc_in{l}", [NLOC, D], F16) for l in range(NL - 1)]
    cc_out = [nc.dram_tensor(f"cc_out{l}", [TB, D], F16, addr_space="Shared")
              for l in range(NL - 1)]
    st_in = [nc.dram_tensor(f"st_in{i}", [D, 2], F) for i in range(2 * NL)]
    st_out = [nc.dram_tensor(f"st_out{i}", [NCORES * D, 2], F,
                             addr_space="Shared") for i in range(2 * NL)]
    rg = [list(range(NCORES))]

    with tile.TileContext(nc) as tc:
        with (
            tc.tile_pool(name="persist", bufs=1) as pp,
            tc.tile_pool(name="gat", bufs=2) as gp,
            tc.tile_pool(name="sg", bufs=4) as sgp,
            tc.tile_pool(name="sb", bufs=2) as sb,
            tc.tile_pool(name="small", bufs=4) as sp,
            tc.tile_pool(name="ps_agg", bufs=2, space="PSUM") as ps_agg,
            tc.tile_pool(name="ps_m", bufs=2, space="PSUM") as ps_m,
            tc.tile_pool(name="ps_m2", bufs=2, space="PSUM") as ps_m2,
            tc.tile_pool(name="ps_tr", bufs=2, space="PSUM") as ps_tr,
        ):
            # resident tiles
            H = pp.tile([D, NLOC], F)
            B1 = pp.tile([D, NLOC], F)
            B2 = pp.tile([D, NLOC], F16)
            idxt = pp.tile([P, NT * BPT * 8], I16)
            wt = pp.tile([P, NL * NT * BPT], F16)
            gb = pp.tile([D, 4 * NL], F)
            ident = pp.tile([D, D], F)

            nc.sync.dma_start(out=H[:], in_=xT_own[:])
            nc.sync.dma_start(out=idxt[:], in_=idx_in[:])
            nc.sync.dma_start(out=wt[:], in_=wt_in[:])
            nc.sync.dma_start(out=gb[:], in_=gb_in[:])
            make_identity(nc, ident[:])
            nc.vector.memset(B1[:], 0.0)
            nc.vector.memset(B2[:], 0.0)
            epsc = pp.tile([D, 1], F)
            nc.vector.memset(epsc[:], BN_EPS)

            for l in range(NL):
                if l == 0:
                    tab_ap = x_table[:]
                else:
                    t_ = cc_out[l - 1][:]
                    tab_ap = bass.AP(t_.tensor, t_.offset,
                                     [[2 * D, TB // 2], [1, 2 * D]])
                W1t = sp.tile([D, D], F16, tag="w1")
                W2t = sp.tile([D, D], F16, tag="w2")
                nc.sync.dma_start(out=W1t[:], in_=Ws_in[2 * l * D:(2 * l + 1) * D, :])
                nc.sync.dma_start(out=W2t[:], in_=Ws_in[(2 * l + 1) * D:(2 * l + 2) * D, :])
                NG = len(_CALLS)
                s1 = sp.tile([D, NG], F, tag="s1")
                s1q = sp.tile([D, NG], F, tag="s1q")

                gpos = 0                              # running block position
                for ci, tiles in enumerate(_CALLS):
                    ntl = len(tiles)
                    t0 = tiles[0]
                    nb = ntl * BPT
                    g0 = gpos
                    gpos += nb
                    nev = ntl * BEV
                    glo = gp.tile([P, G * BPT, 2 * D], F16, tag="glo")
                    if stage < 1:
                        continue
                    Sg = sgp.tile([P, G * BPT, P], F8, tag="Sg")
                    nc.sync.dma_start(
                        out=Sg[:, :nb, :],
                        in_=S_in[:, g0 * P:(g0 + nb) * P])
                    nc.gpsimd.dma_gather(
                        out_ap=glo[:, :nb, :], in_ap=tab_ap,
                        idxs_ap=idxt[:, g0 * 8:(g0 + nb) * 8],
                        num_idxs=nb * P, num_idxs_reg=nb * P,
                        elem_size=2 * D, single_packet=False)
                    if stage < 2:
                        continue
                    # weight messages: evens use token half [0:64], odds
                    # [64:128]; block order per group is evens-of-tiles then
                    # odds-of-tiles, so both are static slices.
                    glob = gp.tile([P, G * BPT, D], F16, tag="glob")
                    wrow = (l * NT) * BPT + g0
                    nc.vector.tensor_tensor(
                        out=glob[:, :nev, :],
                        in0=glo[:, :nev, 0:D],
                        in1=wt[:, wrow:wrow + nev].to_broadcast([P, nev, D]),
                        op=AL.mult)
                    nc.vector.tensor_tensor(
                        out=glob[:, nev:nb, :],
                        in0=glo[:, nev:nb, D:2 * D],
                        in1=wt[:, wrow + nev:wrow + nb]
                        .to_broadcast([P, nb - nev, D]),
                        op=AL.mult)

                    if stage < 3:
                        continue
                    if stage < 4:
                        continue
                    paG = ps_agg.tile([D, G * P], F, space="PSUM", tag="pa")
                    for ti, t in enumerate(tiles):
                        blocks = [ti * BEV + b for b in range(BEV)] + \
                                 [nev + ti * BOD + b for b in range(BOD)]
                        for k, pb in enumerate(blocks):
                            nc.tensor.matmul(
                                out=paG[:, ti * P:(ti + 1) * P],
                                lhsT=glob[:, pb, :],
                                rhs=Sg[:, pb, :],
                                start=(k == 0), stop=(k == BPT - 1))
                    # combine + MLP1, one shot per 4-tile group (pads are
                    # exactly zero in H and agg, so stats over them are safe)
                    y0b = sb.tile([D, G * P], F16, tag="y0")
                    nc.vector.tensor_tensor(
                        out=y0b[:, :ntl * P], in0=paG[:, :ntl * P],
                        in1=H[:, t0 * P:(t0 + ntl) * P], op=AL.add)
                    pmG = ps_m.tile([D, G * P], F, space="PSUM", tag="pm")
                    nc.tensor.matmul(out=pmG[:, :ntl * P], lhsT=W1t[:],
                                     rhs=y0b[:, :ntl * P],
                                     start=True, stop=True)
                    sqG = sb.tile([D, G * P], F, tag="sq")
                    nc.scalar.activation(
                        out=B1[:, t0 * P:(t0 + ntl) * P],
                        in_=pmG[:, :ntl * P],
                        func=AF.Copy, accum_out=s1[:, ci:ci + 1])
                    nc.scalar.activation(
                        out=sqG[:, :ntl * P], in_=pmG[:, :ntl * P],
                        func=AF.Square, accum_out=s1q[:, ci:ci + 1])

                if stage < 5:
                    continue
                # BN stats allreduce (AllGather of [64,2] partials + reduce)
                def bn_stats(sums, sq_t, idx):
                    red = sp.tile([D, 2], F, tag="red")
                    nc.vector.tensor_reduce(out=red[:, 0:1], in_=sums[:],
                                            axis=mybir.AxisListType.X,
                                            op=AL.add)
                    nc.vector.tensor_reduce(out=red[:, 1:2], in_=sq_t[:],
                                            axis=mybir.AxisListType.X,
                                            op=AL.add)
                    nc.sync.dma_start(out=st_in[idx][:], in_=red[:])
                    nc.gpsimd.collective_compute(
                        "AllGather", AL.bypass, replica_groups=rg,
                        ins=[st_in[idx][:]], outs=[st_out[idx][:]])
                    st8 = sp.tile([D, NCORES, 2], F, tag="st8")
                    full_ap = st_out[idx][:]
                    nc.sync.dma_start(
                        out=st8[:],
                        in_=bass.AP(full_ap.tensor, full_ap.offset,
                                    [[2, D], [2 * D, NCORES], [1, 2]]))
                    st = sp.tile([D, 2], F, tag="st")
                    nc.vector.tensor_reduce(out=st[:, 0:1], in_=st8[:, :, 0:1],
                                            axis=mybir.AxisListType.XY,
                                            op=AL.add)
                    nc.vector.tensor_reduce(out=st[:, 1:2], in_=st8[:, :, 1:2],
                                            axis=mybir.AxisListType.XY,
                                            op=AL.add)
                    mean = sp.tile([D, 1], F, tag="mean")
                    ex2 = sp.tile([D, 1], F, tag="ex2")
                    nc.scalar.activation(out=mean[:], in_=st[:, 0:1],
                                         func=AF.Copy, scale=1.0 / NODES)
                    nc.scalar.activation(out=ex2[:], in_=st[:, 1:2],
                                         func=AF.Copy, scale=1.0 / NODES)
                    var = sp.tile([D, 1], F, tag="var")
                    nc.vector.tensor_tensor(out=var[:], in0=mean[:],
                                            in1=mean[:], op=AL.mult)
                    nc.vector.tensor_tensor(out=var[:], in0=ex2[:],
                                            in1=var[:], op=AL.subtract)
                    nc.vector.tensor_tensor(out=var[:], in0=var[:],
                                            in1=epsc[:], op=AL.add)
                    std = sp.tile([D, 1], F, tag="std")
                    nc.scalar.activation(out=std[:], in_=var[:], func=AF.Sqrt,
                                         bias=0.0)
                    rstd = sp.tile([D, 1], F, tag="rstd")
                    nc.vector.reciprocal(rstd[:], std[:])
                    gcol = 4 * l + (0 if idx % 2 == 0 else 2)
                    scl = sp.tile([D, 1], F, tag="scl")
                    nc.vector.tensor_tensor(out=scl[:], in0=gb[:, gcol:gcol + 1],
                                            in1=rstd[:], op=AL.mult)
                    tmp = sp.tile([D, 1], F, tag="tmp")
                    nc.vector.tensor_tensor(out=tmp[:], in0=mean[:],
                                            in1=scl[:], op=AL.mult)
                    bia = sp.tile([D, 1], F, tag="bia")
                    nc.vector.tensor_tensor(out=bia[:],
                                            in0=gb[:, gcol + 1:gcol + 2],
                                            in1=tmp[:], op=AL.subtract)
                    return scl, bia

                sc1, bi1 = bn_stats(s1, s1q, 2 * l)

                # y1n = relu(BN1(y1)) in bf16; y2 = W2.T @ y1n, stats
                s2 = sp.tile([D, 16], F, tag="s2")
                s2q = sp.tile([D, 16], F, tag="s2q")
                nch = (NLOC + CHUNK - 1) // CHUNK
                for ci in range(nch):
                    c0 = ci * CHUNK
                    c1 = min(c0 + CHUNK, NLOC)
                    ca = min(c1, NSH)                # apply-BN limit
                    if ca > c0:
                        nc.scalar.activation(
                            out=B2[:, c0:ca], in_=B1[:, c0:ca], func=AF.Relu,
                            bias=bi1[:], scale=sc1[:])
                    pm2 = ps_m2.tile([D, CHUNK], F, space="PSUM", tag="pm2")
                    nc.tensor.matmul(out=pm2[:, :c1 - c0], lhsT=W2t[:],
                                     rhs=B2[:, c0:c1], start=True, stop=True)
                    sq2 = sb.tile([D, CHUNK], F, tag="sq2")
                    nc.scalar.activation(
                        out=B1[:, c0:c1], in_=pm2[:, :c1 - c0], func=AF.Copy,
                        accum_out=s2[:, ci:ci + 1])
                    # square+reduce on DVE — ACT is this phase's bottleneck;
                    # both ops are patterns already exercised elsewhere here
                    nc.vector.tensor_tensor(
                        out=sq2[:, :c1 - c0], in0=B1[:, c0:c1],
                        in1=B1[:, c0:c1], op=AL.mult)
                    nc.vector.tensor_reduce(
                        out=s2q[:, ci:ci + 1], in_=sq2[:, :c1 - c0],
                        axis=mybir.AxisListType.X, op=AL.add)

                if l == NL - 1:
                    # final layer: ship raw y2 + local stats; host applies
                    # BN2+ReLU to just the masked rows (saves one collective)
                    red2 = sp.tile([D, 2], F, tag="red2")
                    nc.vector.tensor_reduce(out=red2[:, 0:1],
                                            in_=s2[:, :nch],
                                            axis=mybir.AxisListType.X,
                                            op=AL.add)
                    nc.vector.tensor_reduce(out=red2[:, 1:2],
                                            in_=s2q[:, :nch],
                                            axis=mybir.AxisListType.X,
                                            op=AL.add)
                    nc.sync.dma_start(out=st2_out[:], in_=red2[:])
                    nc.scalar.activation(out=H[:], in_=B1[:], func=AF.Copy)
                else:
                    sc2, bi2 = bn_stats(s2[:, :nch], s2q[:, :nch], 2 * l + 1)

                    # h_next = relu(BN2(y2)), transpose, store / allgather.
                    for tiles2 in _CALLS:
                        ntl2, t0b = len(tiles2), tiles2[0]
                        nc.scalar.activation(
                            out=H[:, t0b * P:(t0b + ntl2) * P],
                            in_=B1[:, t0b * P:(t0b + ntl2) * P],
                            func=AF.Relu, bias=bi2[:], scale=sc2[:])
                    nc.vector.memset(H[:, NSH:NLOC], 0.0)
                dst = h_out if l == NL - 1 else cc_in[l]
                for tiles2 in _CALLS:
                    ntl2, t0b = len(tiles2), tiles2[0]
                    ptrG = ps_tr.tile([P, G, D], F, space="PSUM", tag="ptr")
                    for k, t in enumerate(tiles2):
                        nc.tensor.transpose(out=ptrG[:, k, :],
                                            in_=H[:, t * P:(t + 1) * P],
                                            identity=ident[:])
                    if l == NL - 1:
                        stgG = sb.tile([P, G, D], F, tag="stgf")
                    else:
                        stgG = sb.tile([P, G, D], F16, tag="stgb")
                    nc.scalar.activation(out=stgG[:, :ntl2, :],
                                         in_=ptrG[:, :ntl2, :], func=AF.Copy)
                    # one store per 4-tile group: HWDGE fixed cost dominates
                    # per-tile stores; AP shape matches the verified st8 read
                    d_ = dst[:]
                    dap = bass.AP(d_.tensor, t0b * P * D,
                                  [[D, P], [P * D, ntl2], [1, D]])
                    nc.sync.dma_start(out=dap, in_=stgG[:, :ntl2, :])
                if l < NL - 1:
                    nc.gpsimd.collective_compute(
                        "AllGather", AL.bypass, replica_groups=rg,
                        ins=[cc_in[l][:]], outs=[cc_out[l][:]])
            if stage < 5:
                nc.sync.dma_start(out=h_out[0:D, :], in_=H[:, 0:D])

    nc.compile()
    return nc


def _get_nc(BEV, BOD):
    if (BEV, BOD) not in _cache:
        _cache[(BEV, BOD)] = _build(BEV, BOD)
    return _cache[(BEV, BOD)]


def _in_maps(plan, table, Ws, gb):
    maps = []
    tbf = table.astype(H_NP).reshape(TB // 2, 2 * D)
    Wsb = Ws.astype(H_NP)
    for c in range(NCORES):
        pc = plan["cores"][c]
        maps.append({
            "x_table": tbf,
            "xT_own": table[c * NLOC:(c + 1) * NLOC].T.copy(),
            "idx": pc["idx"], "wt": pc["wt"], "S": pc["S"],
            "Ws": Wsb, "gb": gb,
        })
    return maps


def kernel(x, edge_index, edge_weight, mask_teams, eps, W1, b1, g1, beta1,
           W2, b2, g2, beta2, _trace=False):
    from concourse.bass_utils import run_bass_kernel_spmd

    x = np.asarray(x, np.float32)
    eps = np.asarray(eps, np.float32)
    plan = _plan(np.asarray(edge_index), np.asarray(edge_weight), eps)
    table, _ = _tableize(x)
    Ws, gb = _weights(eps, np.asarray(W1), np.asarray(W2), np.asarray(g1),
                      np.asarray(beta1), np.asarray(g2), np.asarray(beta2))
    in_maps = _in_maps(plan, table, Ws, gb)

    nc = _get_nc(plan["BEV"], plan["BOD"])
    res = run_bass_kernel_spmd(nc, in_maps, list(range(NCORES)), trace=_trace)
    full = np.concatenate([res.results[c]["h_out"][:NSH]
                           for c in range(NCORES)], 0)
    # final-layer BN2 + ReLU on the host (device ships raw y2 + stats)
    st2 = sum(res.results[c]["st2"].astype(np.float64)
              for c in range(NCORES))
    mu2 = st2[:, 0] / NODES
    var2 = st2[:, 1] / NODES - mu2 ** 2
    lf = NL - 1
    sc2 = gb[:, 4 * lf + 2] / np.sqrt(var2 + BN_EPS)
    bi2 = gb[:, 4 * lf + 3] - mu2 * sc2
    y2m = full[np.asarray(mask_teams)].astype(np.float64)
    out = np.maximum(y2m * sc2[None, :] + bi2[None, :], 0).astype(np.float32)
    if _trace:
        kernel._last = res
    return out
